# revision 1
# baseline (speedup 1.0000x reference)
"""Fused RoPE attention + LayerNorm, Trainium2, 8 NeuronCores (SPMD).

Sharding: every core takes the same 256-position slice of BOTH batches
(512 q-rows/core).  K/V projections are computed for the local rows,
all-gathered (bf16) across the 8 cores, then each core runs full
attention + LayerNorm for its rows.  All post-gather addressing is
core-independent, as required by the single-program SPMD model.

RoPE is applied without cross-partition shuffles: rot2(U) is a signed
pair-permutation, computed on the TensorEngine as perm^T @ U with a
constant [128,128] matrix; then q_rot = U*cos + rot2(U)*sin on DVE.
"""
import sys
import types
import os
import numpy as np
from contextlib import ExitStack

for _p in ("/opt/trn_rl_repo",):
    if _p not in sys.path:
        sys.path.append(_p)

# NTFF profile hook shim: lets BASS_TRACE=1 work in images whose antenv
# lacks axon_hooks (bass_utils imports it when tracing under axon).
if "antenv.axon_hooks" not in sys.modules:
    _hooks = types.ModuleType("antenv.axon_hooks")
    _HOOK = [None]
    _hooks.set_axon_ntff_profile_hook = lambda h: _HOOK.__setitem__(0, h)
    _hooks.get_axon_ntff_profile_hook = lambda: _HOOK[0]
    sys.modules["antenv.axon_hooks"] = _hooks
    try:
        from trn_agent_boot.trn_boot import _ntff_profile_via_ctypes

        _HOOK[0] = _ntff_profile_via_ctypes("/opt/axon/libaxon_pjrt.so")
    except Exception:
        pass

import concourse.bass as bass  # noqa: E402
import concourse.bacc as bacc  # noqa: E402
import concourse.mybir as mybir  # noqa: E402
import concourse.tile as tile  # noqa: E402
from concourse import bass_utils  # noqa: E402

F32 = mybir.dt.float32
BF16 = mybir.dt.bfloat16
NP_BF16 = np.dtype(mybir.dt.np(BF16))
AF = mybir.ActivationFunctionType
ALU = mybir.AluOpType
AX = mybir.AxisListType

B, S, D, H, DH = 2, 2048, 1024, 16, 64
NC = 8
SL = S // NC          # 256 positions per core (per batch)
R = B * SL            # 512 rows per core
G = H // 2            # 8 head-pairs
DC = D // 128         # 8 contraction chunks
KT = S // 128         # 16 k-tiles per batch
LN_EPS = 1e-5
ROPE_BASE = 10000.0


def _build(flags):
    has_bqk, has_bv, has_gb = flags
    STAGE = int(os.environ.get("KSTAGE", "4"))
    nc = bacc.Bacc("TRN2", target_bir_lowering=False, debug=False,
                   num_devices=NC)

    xqT = nc.dram_tensor("xqT", [D, R], BF16, kind="ExternalInput")
    xvT = nc.dram_tensor("xvT", [D, R], BF16, kind="ExternalInput")
    wq_d = nc.dram_tensor("wq", [D, D], BF16, kind="ExternalInput")
    wk_d = nc.dram_tensor("wk", [D, D], BF16, kind="ExternalInput")
    wv_d = nc.dram_tensor("wv", [D, D], BF16, kind="ExternalInput")
    perm_d = nc.dram_tensor("perm", [128, 128], BF16, kind="ExternalInput")
    ident_d = nc.dram_tensor("ident", [128, 128], BF16, kind="ExternalInput")
    cos_d = nc.dram_tensor("cos", [128, R], F32, kind="ExternalInput")
    sin_d = nc.dram_tensor("sin", [128, R], F32, kind="ExternalInput")
    if has_bqk:
        cq_d = nc.dram_tensor("cq", [D, R], F32, kind="ExternalInput")
        ck_d = nc.dram_tensor("ck", [D, R], F32, kind="ExternalInput")
    if has_bv:
        bv_d = nc.dram_tensor("bv", [128, D], F32, kind="ExternalInput")
    if has_gb:
        gam_d = nc.dram_tensor("gamma", [128, D], F32, kind="ExternalInput")
        bet_d = nc.dram_tensor("beta", [128, D], F32, kind="ExternalInput")
    out_d = nc.dram_tensor("out", [R, D], F32, kind="ExternalOutput")

    es = ExitStack()
    with es:
        tc = es.enter_context(tile.TileContext(nc))
        dram = es.enter_context(
            tc.tile_pool(name="dram", bufs=1, space="DRAM"))
        constp = es.enter_context(tc.tile_pool(name="const", bufs=1))
        qp = es.enter_context(tc.tile_pool(name="qp", bufs=1))
        kvs = es.enter_context(tc.tile_pool(name="kvs", bufs=2))
        ptp = es.enter_context(tc.tile_pool(name="ptp", bufs=10))
        attnp = es.enter_context(tc.tile_pool(name="attnp", bufs=1))
        epip = es.enter_context(tc.tile_pool(name="epip", bufs=8))
        lnp = es.enter_context(tc.tile_pool(name="lnp", bufs=2))
        outp = es.enter_context(tc.tile_pool(name="outp", bufs=2))

        bounce_k = dram.tile([D, R], BF16, tag="bk")
        ag_k = dram.tile([NC * D, R], BF16, tag="agk", addr_space="Shared")
        bounce_v = dram.tile([R, H * 65], BF16, tag="bv")
        ag_v = dram.tile([NC * R, H * 65], BF16, tag="agv",
                         addr_space="Shared")

        cos_sb = constp.tile([128, R], F32, tag="cos")
        sin_sb = constp.tile([128, R], F32, tag="sin")
        perm_sb = constp.tile([128, 128], BF16, tag="perm")
        ident_sb = constp.tile([128, 128], BF16, tag="ident")
        eps_sb = constp.tile([128, 1], F32, tag="eps")
        nc.vector.memset(eps_sb[:], LN_EPS)
        cq_sb = ck_sb = bv_sb = gam_sb = bet_sb = None
        if has_bqk:
            cq_sb = constp.tile([128, DC * R], F32, tag="cq")
            ck_sb = constp.tile([128, DC * R], F32, tag="ck")
            for g in range(G):
                nc.sync.dma_start(cq_sb[:, g * R:(g + 1) * R],
                                  cq_d[g * 128:(g + 1) * 128, :])
                nc.sync.dma_start(ck_sb[:, g * R:(g + 1) * R],
                                  ck_d[g * 128:(g + 1) * 128, :])
        if has_bv:
            bv_sb = constp.tile([128, D], F32, tag="bvs")
            nc.sync.dma_start(bv_sb[:], bv_d[:])
        if has_gb:
            gam_sb = constp.tile([128, D], F32, tag="gam")
            nc.sync.dma_start(gam_sb[:], gam_d[:])
            bet_sb = constp.tile([128, D], F32, tag="bet")
            nc.sync.dma_start(bet_sb[:], bet_d[:])

        q_sb = qp.tile([128, G * R], BF16, tag="qrot")
        q1_sb = qp.tile([64, G * R], BF16, tag="qrot1")

        pes = ExitStack()
        with pes:
            xp = pes.enter_context(tc.tile_pool(name="xp", bufs=1))
            wp = pes.enter_context(tc.tile_pool(name="wp", bufs=2))
            stage = pes.enter_context(tc.tile_pool(name="stage", bufs=3))
            usbp = pes.enter_context(tc.tile_pool(name="usbp", bufs=3))
            krotp = pes.enter_context(tc.tile_pool(name="krotp", bufs=2))
            vstp = pes.enter_context(tc.tile_pool(name="vstp", bufs=2))
            pjp = pes.enter_context(
                tc.tile_pool(name="pjp", bufs=4, space="PSUM"))
            pvp = pes.enter_context(
                tc.tile_pool(name="pvp", bufs=2, space="PSUM"))

            def load_w(t_dram):
                w_sb = wp.tile([128, DC * D], BF16, tag="w")
                for dc in range(DC):
                    nc.sync.dma_start(w_sb[:, dc * D:(dc + 1) * D],
                                      t_dram[dc * 128:(dc + 1) * 128, :])
                return w_sb

            # first projection depends only on chunk 0 of wk/xq -- emit
            # those first so the PE starts as early as possible.
            wk_sb = load_w(wk_d)
            xq_sb = xp.tile([128, DC * R], BF16, tag="xq")
            for dc in range(DC):
                nc.sync.dma_start(xq_sb[:, dc * R:(dc + 1) * R],
                                  xqT[dc * 128:(dc + 1) * 128, :])
            nc.sync.dma_start(perm_sb[:], perm_d[:])
            nc.sync.dma_start(cos_sb[:], cos_d[:])
            nc.sync.dma_start(sin_sb[:], sin_d[:])
            nc.sync.dma_start(ident_sb[:], ident_d[:])
            xv_sb = xp.tile([128, DC * R], BF16, tag="xv")
            for dc in range(DC):
                nc.sync.dma_start(xv_sb[:, dc * R:(dc + 1) * R],
                                  xvT[dc * 128:(dc + 1) * 128, :])

            # --- Q/K projection, software-pipelined so the perm matmul of
            # group g runs behind the U matmuls of group g+1 and the PE
            # never waits on the ScalarE psum->sbuf copy. ---
            def proj_u(w_sb, g):
                ps_u = pjp.tile([128, R], F32, tag="pj", name=f"psu{id(w_sb)}_{g}")
                for dc in range(DC):
                    nc.tensor.matmul(
                        ps_u[:],
                        w_sb[:, dc * D + g * 128: dc * D + (g + 1) * 128],
                        xq_sb[:, dc * R:(dc + 1) * R],
                        start=(dc == 0), stop=(dc == DC - 1))
                u_sb = usbp.tile([128, R], BF16, tag="usb",
                                 name=f"usb{id(w_sb)}_{g}")
                nc.scalar.copy(u_sb[:], ps_u[:])
                return ps_u, u_sb

            def proj_rope(g, ps_u, u_sb, c_sb, dst_bf):
                ps_u2 = pjp.tile([128, R], F32, tag="pj", name=f"psu2_{g}")
                nc.tensor.matmul(ps_u2[:], perm_sb[:], u_sb[:],
                                 start=True, stop=True)
                t1 = stage.tile([128, R], F32, tag="st", name=f"t1_{g}")
                nc.vector.tensor_mul(t1[:], ps_u[:], cos_sb[:])
                t2 = stage.tile([128, R], F32, tag="st", name=f"t2_{g}")
                nc.vector.tensor_mul(t2[:], ps_u2[:], sin_sb[:])
                if c_sb is None:
                    nc.vector.tensor_add(dst_bf, t1[:], t2[:])
                else:
                    t3 = stage.tile([128, R], F32, tag="st", name=f"t3_{g}")
                    nc.vector.tensor_add(t3[:], t1[:], t2[:])
                    nc.vector.tensor_add(
                        dst_bf, t3[:], c_sb[:, g * R:(g + 1) * R])

            def qk_proj_all(w_sb, c_sb, emit_dst, tail, groups):
                pend = None
                for g in groups:
                    cur = (g,) + proj_u(w_sb, g)
                    if pend is not None:
                        gp = pend[0]
                        proj_rope(*pend, c_sb, emit_dst(gp))
                        tail(gp)
                    pend = cur
                gp = pend[0]
                proj_rope(*pend, c_sb, emit_dst(gp))
                tail(gp)

            # K projection + RoPE -> bounce, AllGather
            krots = {}

            def k_dst(g):
                krots[g] = krotp.tile([128, R], BF16, tag="kr",
                                      name=f"kr{g}")
                return krots[g][:]

            def k_tail(g):
                nc.sync.dma_start(
                    bounce_k[g * 128:(g + 1) * 128, :], krots[g][:])

            qk_proj_all(wk_sb, ck_sb, k_dst, k_tail, range(4))
            qk_proj_all(wk_sb, ck_sb, k_dst, k_tail, range(4, G))
            nc.gpsimd.collective_compute(
                "AllGather", ALU.bypass,
                ins=[bounce_k[:].opt()], outs=[ag_k[:].opt()],
                replica_groups=[list(range(NC))])

            # V projection -> bounce (ones interleaved), AllGather
            wv_sb = load_w(wv_d)
            for st in range(R // 128):
                ps_v = pvp.tile([128, D], F32, tag="pv")
                for dc in range(DC):
                    for hf in range(2):
                        nc.tensor.matmul(
                            ps_v[:, hf * 512:(hf + 1) * 512],
                            xv_sb[:, dc * R + st * 128:
                                  dc * R + st * 128 + 128],
                            wv_sb[:, dc * D + hf * 512:
                                  dc * D + (hf + 1) * 512],
                            start=(dc == 0), stop=(dc == DC - 1))
                if has_bv:
                    nc.vector.tensor_add(ps_v[:], ps_v[:], bv_sb[:])
                v_sb = vstp.tile([128, H * 65], BF16, tag="vst")
                v3 = v_sb[:].rearrange("p (h e) -> p h e", e=65)
                nc.vector.memset(v3[:, :, 64:65], 1.0)
                nc.scalar.copy(
                    v3[:, :, 0:64],
                    ps_v[:].rearrange("p (h d) -> p h d", d=64))
                nc.sync.dma_start(
                    bounce_v[st * 128:(st + 1) * 128, :], v_sb[:])
            nc.gpsimd.collective_compute(
                "AllGather", ALU.bypass,
                ins=[bounce_v[:].opt()], outs=[ag_v[:].opt()],
                replica_groups=[list(range(NC))])


            # Q projection + RoPE (stays local).  The odd head's half is
            # copied to partitions 0-63 (base-partition-64 matmul operands
            # hang this hardware).
            wq_sb = load_w(wq_d)

            def q_dst(g):
                return q_sb[:, g * R:(g + 1) * R]

            def q_tail(g):
                nc.sync.dma_start(q1_sb[:, g * R:(g + 1) * R],
                                  q_sb[64:128, g * R:(g + 1) * R])

            qk_proj_all(wq_sb, cq_sb, q_dst, q_tail, range(G))

        # V resident for the whole attention phase: [s-tile, 16 heads x 65]
        # per (batch, k-tile), contiguous 2080-byte lines.
        vfp = es.enter_context(tc.tile_pool(name="vfp", bufs=1))
        v_full = vfp.tile([128, B * KT * H * 65], BF16, tag="vfull")
        for b in range(B):
            for kt in range(KT):
                base = (kt // 2) * R + b * SL + (kt % 2) * 128
                nc.sync.dma_start(
                    v_full[:, (b * KT + kt) * (H * 65):
                           (b * KT + kt + 1) * (H * 65)],
                    ag_v[base: base + 128, :])

        attn_sb = [attnp.tile([128, D], F32, tag=f"attn{t}", name=f"attn{t}")
                   for t in range(4)] if STAGE >= 2 else []
        GLIM = G if STAGE >= 3 else (1 if STAGE == 2 else 0)

        aes = ExitStack()
        with aes:
            scp = aes.enter_context(
                tc.tile_pool(name="scp", bufs=2, space="PSUM"))
            avp = aes.enter_context(
                tc.tile_pool(name="avp", bufs=2, space="PSUM"))

            trp = aes.enter_context(
                tc.tile_pool(name="trp", bufs=2, space="PSUM"))
            atsb = aes.enter_context(tc.tile_pool(name="atsb", bufs=3))

            kph_all = {}

            def load_pair(g):
                kph = [kvs.tile([64, B * S], BF16, tag=f"kp{_h}",
                                name=f"kp{g}_{_h}") for _h in range(2)]
                for hh in range(2):
                    kp3 = kph[hh][:].rearrange("p (b t) -> p b t", b=B)
                    for r in range(NC):
                        srcap = ag_k[r * D + g * 128 + hh * 64:
                                     r * D + g * 128 + hh * 64 + 64, :]
                        nc.sync.dma_start(
                            kp3[:, :, r * SL:(r + 1) * SL],
                            srcap.rearrange("p (b w) -> p b w", b=B))
                kph_all[g] = kph

            def emit_scores_exp(g, b):
                kph = kph_all[g]
                pts = []
                for grp in range(4):
                    ps_s = [scp.tile([128, 1024], F32, tag="sc",
                                     name=f"pss{g}_{b}_{grp}_{_i}")
                            for _i in range(2)]
                    for jj in range(4):
                        kt = grp * 4 + jj
                        for hh in range(2):
                            qsrc = q_sb if hh == 0 else q1_sb
                            nc.tensor.matmul(
                                ps_s[hh][:, jj * SL:(jj + 1) * SL],
                                kph[hh][0:64,
                                        b * S + kt * 128:
                                        b * S + (kt + 1) * 128],
                                qsrc[0:64,
                                     g * R + b * SL:
                                     g * R + (b + 1) * SL],
                                start=True, stop=True)
                    for hh in range(2):
                        pt = ptp.tile([128, 1024], BF16, tag="pt",
                                      name=f"pt{g}_{b}_{grp}_{hh}")
                        nc.scalar.activation(
                            pt[:], ps_s[hh][:], AF.Exp, scale=0.125)
                        pts.append(((grp, hh), pt))
                return dict(pts)

            def emit_av(g, b, pts):
                # attn^T accumulation: out[65, 256] = [V_h | 1]^T @ P^T,
                # then PE-transpose back to [q, dh] and normalize.
                aTp = avp.tile([65, 2 * SL], F32, tag="av",
                               name=f"aT{g}_{b}")
                for hh in range(2):
                    h = 2 * g + hh
                    aT = aTp[:, hh * SL:(hh + 1) * SL]
                    for grp in range(4):
                        for jj in range(4):
                            kt = grp * 4 + jj
                            nc.tensor.matmul(
                                aT,
                                v_full[:, (b * KT + kt) * (H * 65)
                                       + h * 65:
                                       (b * KT + kt) * (H * 65)
                                       + (h + 1) * 65],
                                pts[(grp, hh)][:, jj * SL:(jj + 1) * SL],
                                start=(kt == 0), stop=(kt == 15))
                    aT_sb = atsb.tile([65, SL], BF16, tag="ats",
                                      name=f"ats{g}_{b}_{hh}")
                    nc.vector.tensor_copy(aT_sb[:], aT)
                    tr = trp.tile([128, 132], BF16, tag="tr",
                                  name=f"tr{g}_{b}_{hh}")
                    for t in range(2):
                        nc.tensor.transpose(
                            tr[:, t * 66: t * 66 + 65],
                            aT_sb[:, t * 128:(t + 1) * 128],
                            ident_sb[0:65, 0:65])
                    rec = epip.tile([128, 2], F32, tag="rec",
                                    name=f"rec{g}_{b}_{hh}")
                    nc.vector.reciprocal(rec[:], tr[:, 64::66])
                    for t in range(2):
                        qtg = b * 2 + t
                        nc.vector.tensor_scalar(
                            attn_sb[qtg][:, h * 64:(h + 1) * 64],
                            tr[:, t * 66: t * 66 + 64],
                            rec[:, t: t + 1], None, ALU.mult)

            # one-block software pipeline: AV of block i runs behind
            # scores/exp of block i+1 so the PE never stalls on the exp.
            blocks = [(g, b) for g in range(GLIM) for b in range(B)]
            pend = None
            for (g, b) in blocks:
                if b == 0:
                    load_pair(g)
                pts = emit_scores_exp(g, b)
                if pend is not None:
                    emit_av(*pend)
                pend = (g, b, pts)
            if pend is not None:
                emit_av(*pend)

        # --- LayerNorm over D (var = E[x^2] - mu^2; square+row-sum on the
        # ScalarE accumulator) + store ---
        for qtg in range(4 if STAGE >= 2 else 0):
            at = attn_sb[qtg]
            if STAGE <= 3:
                o_sb = outp.tile([128, D], F32, tag="o", name=f"o{qtg}")
                nc.vector.tensor_copy(o_sb[:], at[:])
                nc.sync.dma_start(out_d[qtg * 128:(qtg + 1) * 128, :],
                                  o_sb[:])
                continue
            sums = epip.tile([128, 1], F32, tag="s1", name=f"s1_{qtg}")
            nc.vector.reduce_sum(sums[:], at[:], axis=AX.X)
            sq = lnp.tile([128, D], F32, tag="sq", name=f"sq{qtg}")
            ssum = epip.tile([128, 1], F32, tag="s3", name=f"s3_{qtg}")
            nc.scalar.activation(sq[:], at[:], AF.Square,
                                 accum_out=ssum[:])
            mu = epip.tile([128, 1], F32, tag="s2", name=f"s2_{qtg}")
            nc.vector.tensor_scalar_mul(mu[:], sums[:], 1.0 / D)
            var = epip.tile([128, 1], F32, tag="s6", name=f"s6_{qtg}")
            nc.vector.tensor_scalar(var[:], mu[:], mu[:], None, ALU.mult)
            nc.vector.tensor_scalar(
                var[:], var[:], -1.0, None, ALU.mult)
            nc.vector.scalar_tensor_tensor(
                var[:], ssum[:], 1.0 / D, var[:], ALU.mult, ALU.add)
            std = epip.tile([128, 1], F32, tag="s4", name=f"s4_{qtg}")
            nc.scalar.activation(std[:], var[:], AF.Sqrt, bias=eps_sb[:])
            rstd = epip.tile([128, 1], F32, tag="s5", name=f"s5_{qtg}")
            nc.vector.reciprocal(rstd[:], std[:])
            mrs = epip.tile([128, 1], F32, tag="s7", name=f"s7_{qtg}")
            nc.vector.tensor_scalar(mrs[:], mu[:], rstd[:], None, ALU.mult)
            o_sb = outp.tile([128, D], F32, tag="o", name=f"oo{qtg}")
            nc.vector.tensor_scalar(
                o_sb[:], at[:], rstd[:], mrs[:], ALU.mult, ALU.subtract)
            if has_gb:
                nc.vector.tensor_mul(o_sb[:], o_sb[:], gam_sb[:])
                nc.vector.tensor_add(o_sb[:], o_sb[:], bet_sb[:])
            nc.sync.dma_start(out_d[qtg * 128:(qtg + 1) * 128, :], o_sb[:])

    nc.compile()
    return nc


_CACHE: dict = {}
LAST_EXEC_NS = None


def _rope_tables():
    half = DH // 2
    inv_freq = 1.0 / (ROPE_BASE ** (np.arange(half, dtype=np.float32) / half))
    t = np.arange(S, dtype=np.float32)
    freqs = t[:, None] * inv_freq[None, :]
    emb = np.concatenate([freqs, freqs], axis=-1)          # [S, DH]
    return np.cos(emb).astype(np.float32), np.sin(emb).astype(np.float32)


def prep_flags(inputs):
    b_qk = np.asarray(inputs["b_qk"], dtype=np.float32)
    b_v = np.asarray(inputs["b_v"], dtype=np.float32)
    gamma = np.asarray(inputs["ln_gamma"], dtype=np.float32)
    beta = np.asarray(inputs["ln_beta"], dtype=np.float32)
    return (bool(np.any(b_qk)), bool(np.any(b_v)),
            bool(np.any(gamma != 1.0) or np.any(beta != 0.0)))


def prep_in_maps(inputs):
    flags = prep_flags(inputs)
    return _prep_in_maps(inputs, flags)


def _prep_in_maps(inputs, flags):
    x_qk = np.asarray(inputs["x_qk"], dtype=np.float32)
    x_v = np.asarray(inputs["x_v"], dtype=np.float32)
    W_qk = np.asarray(inputs["W_qk"], dtype=np.float32)
    b_qk = np.asarray(inputs["b_qk"], dtype=np.float32)
    W_v = np.asarray(inputs["W_v"], dtype=np.float32)
    b_v = np.asarray(inputs["b_v"], dtype=np.float32)
    gamma = np.asarray(inputs["ln_gamma"], dtype=np.float32)
    beta = np.asarray(inputs["ln_beta"], dtype=np.float32)

    # signed pair-swap: rot2(v)[j] = sum_l Pm[l, j] v[l]
    Pm = np.zeros((128, 128), np.float32)
    for i in range(64):
        Pm[2 * i + 1, 2 * i] = -1.0
        Pm[2 * i, 2 * i + 1] = 1.0
    Pm64 = Pm[:DH, :DH]

    cos_all, sin_all = _rope_tables()
    Wq = W_qk[:, :D]
    Wk = W_qk[:, D:]
    bq = b_qk[:D]
    bk = b_qk[D:]
    bq2 = (bq.reshape(H, DH) @ Pm64).reshape(D)
    bk2 = (bk.reshape(H, DH) @ Pm64).reshape(D)

    wq_np = np.ascontiguousarray(Wq.astype(NP_BF16))
    wk_np = np.ascontiguousarray(Wk.astype(NP_BF16))
    wv_np = np.ascontiguousarray(W_v.astype(NP_BF16))
    perm_np = np.ascontiguousarray(Pm.astype(NP_BF16))

    xf = x_qk.reshape(B * S, D)
    xvf = x_v.reshape(B * S, D)

    in_maps = []
    for c in range(NC):
        ps = np.arange(SL * c, SL * (c + 1))
        rows = np.concatenate([ps, S + ps])          # both batches
        xqT_c = np.ascontiguousarray(xf[rows].T.astype(NP_BF16))
        xvT_c = np.ascontiguousarray(xvf[rows].T.astype(NP_BF16))
        cos_c = np.ascontiguousarray(np.tile(cos_all[ps].T, (2, 2)))
        sin_c = np.ascontiguousarray(np.tile(sin_all[ps].T, (2, 2)))
        m = {
            "xqT": xqT_c, "xvT": xvT_c,
            "wq": wq_np, "wk": wk_np, "wv": wv_np,
            "perm": perm_np, "cos": cos_c, "sin": sin_c,
            "ident": np.ascontiguousarray(np.eye(128, dtype=NP_BF16)),
        }
        if flags[0]:
            cos_f = np.tile(cos_all[ps].T, (H, 2))   # [1024, 512]
            sin_f = np.tile(sin_all[ps].T, (H, 2))
            m["cq"] = np.ascontiguousarray(
                bq[:, None] * cos_f + bq2[:, None] * sin_f)
            m["ck"] = np.ascontiguousarray(
                bk[:, None] * cos_f + bk2[:, None] * sin_f)
        if flags[1]:
            m["bv"] = np.ascontiguousarray(
                np.broadcast_to(b_v, (128, D)).astype(np.float32))
        if flags[2]:
            m["gamma"] = np.ascontiguousarray(
                np.broadcast_to(gamma, (128, D)).astype(np.float32))
            m["beta"] = np.ascontiguousarray(
                np.broadcast_to(beta, (128, D)).astype(np.float32))
        in_maps.append(m)
    return in_maps


def assemble_output(per_core_outs):
    out = np.empty((B * S, D), np.float32)
    for c in range(NC):
        oc = np.asarray(per_core_outs[c], dtype=np.float32)
        for b in range(B):
            out[b * S + SL * c: b * S + SL * (c + 1)] = \
                oc[b * SL:(b + 1) * SL]
    return out.reshape(B, S, D)


def kernel(**inputs):
    flags = prep_flags(inputs)
    if flags not in _CACHE:
        _CACHE[flags] = _build(flags)
    nc = _CACHE[flags]
    in_maps = _prep_in_maps(inputs, flags)
    res = bass_utils.run_bass_kernel_spmd(
        nc, in_maps, core_ids=list(range(NC)))
    global LAST_EXEC_NS
    LAST_EXEC_NS = res.exec_time_ns
    return assemble_output([res.results[c]["out"] for c in range(NC)])



# revision 8
# speedup vs baseline: 1.0625x; 1.0625x over previous
"""Fused RoPE attention + LayerNorm, Trainium2, 8 NeuronCores (SPMD).

Head-parallel sharding: core c owns head pair (2c, 2c+1) and computes
Q/K/V projections + attention for the FULL sequence (both batches) for
its two heads.  Inputs x_qk / x_v are replicated to every core (DMA,
not collectives), so the big K/V AllGathers of the row-sharded design
disappear.  The only collective is a tiny LayerNorm-stats AllReduce
(each core holds 128 of the 1024 columns of attn output; row mean/var
need all 1024), overlapped with attention of the other batch.

RoPE is applied without cross-partition shuffles: rot2(U) is a signed
pair-permutation, computed on the TensorEngine as perm^T @ U with a
constant [128,128] matrix; then q_rot = U*cos + rot2(U)*sin (DVE+Pool).

Engine split: PE matmuls; Act psum->sbuf proj copies + exp + LN sqrt;
DVE RoPE muls/adds, aT casts, recip, stats, LN math+normalize;
Pool (gpsimd) RoPE sin-mul, V pack copies, attn normalize.
"""
import sys
import types
import os
import numpy as np
from contextlib import ExitStack

for _p in ("/opt/trn_rl_repo",):
    if _p not in sys.path:
        sys.path.append(_p)

# NTFF profile hook shim: lets BASS_TRACE=1 work in images whose antenv
# lacks axon_hooks (bass_utils imports it when tracing under axon).
if "antenv.axon_hooks" not in sys.modules:
    _hooks = types.ModuleType("antenv.axon_hooks")
    _HOOK = [None]
    _hooks.set_axon_ntff_profile_hook = lambda h: _HOOK.__setitem__(0, h)
    _hooks.get_axon_ntff_profile_hook = lambda: _HOOK[0]
    sys.modules["antenv.axon_hooks"] = _hooks
    try:
        from trn_agent_boot.trn_boot import _ntff_profile_via_ctypes

        _HOOK[0] = _ntff_profile_via_ctypes("/opt/axon/libaxon_pjrt.so")
    except Exception:
        pass

import concourse.bass as bass  # noqa: E402
import concourse.bacc as bacc  # noqa: E402
import concourse.mybir as mybir  # noqa: E402
import concourse.tile as tile  # noqa: E402
from concourse import bass_utils  # noqa: E402

F32 = mybir.dt.float32
BF16 = mybir.dt.bfloat16
NP_BF16 = np.dtype(mybir.dt.np(BF16))
AF = mybir.ActivationFunctionType
ALU = mybir.AluOpType
AX = mybir.AxisListType

B, S, D, H, DH = 2, 2048, 1024, 16, 64
NC = 8
R = B * S             # 4096 rows (positions across both batches)
DC = D // 128         # 8 contraction chunks
NSEG = 8              # projection segments of 512 positions
SEGW = R // NSEG      # 512
KT = 32               # global 128-key tiles (16 per batch)
NQB = 4               # 512-wide q blocks per batch
LN_EPS = 1e-5
ROPE_BASE = 10000.0


def _build(flags):
    has_bqk, has_bv, has_gb = flags
    nc = bacc.Bacc("TRN2", target_bir_lowering=False, debug=False,
                   num_devices=NC)

    xqT = nc.dram_tensor("xqT", [D, R], BF16, kind="ExternalInput")
    xvT = nc.dram_tensor("xvT", [D, R], BF16, kind="ExternalInput")
    wq_d = nc.dram_tensor("wq", [D, 128], BF16, kind="ExternalInput")
    wk_d = nc.dram_tensor("wk", [D, 128], BF16, kind="ExternalInput")
    wv_d = nc.dram_tensor("wv", [D, 128], BF16, kind="ExternalInput")
    perm_d = nc.dram_tensor("perm", [128, 128], BF16, kind="ExternalInput")
    ident_d = nc.dram_tensor("ident", [128, 128], BF16, kind="ExternalInput")
    cos_d = nc.dram_tensor("cos", [128, R], F32, kind="ExternalInput")
    sin_d = nc.dram_tensor("sin", [128, R], F32, kind="ExternalInput")
    if has_bqk:
        cq_d = nc.dram_tensor("cq", [128, R], F32, kind="ExternalInput")
        ck_d = nc.dram_tensor("ck", [128, R], F32, kind="ExternalInput")
    if has_bv:
        bv_d = nc.dram_tensor("bv", [128, 128], F32, kind="ExternalInput")
    if has_gb:
        gam_d = nc.dram_tensor("gamma", [128, 128], F32, kind="ExternalInput")
        bet_d = nc.dram_tensor("beta", [128, 128], F32, kind="ExternalInput")
    out_d = nc.dram_tensor("out", [R, 128], F32, kind="ExternalOutput")

    es = ExitStack()
    with es:
        tc = es.enter_context(tile.TileContext(nc))
        dram = es.enter_context(
            tc.tile_pool(name="dram", bufs=1, space="DRAM"))
        constp = es.enter_context(tc.tile_pool(name="const", bufs=1))
        qkp = es.enter_context(tc.tile_pool(name="qkp", bufs=1))
        vfp = es.enter_context(tc.tile_pool(name="vfp", bufs=1))
        ptp = es.enter_context(tc.tile_pool(name="ptp", bufs=2))
        attnp = es.enter_context(tc.tile_pool(name="attnp", bufs=1))
        statp = es.enter_context(tc.tile_pool(name="statp", bufs=1))
        lnp = es.enter_context(tc.tile_pool(name="lnp", bufs=2))
        outp = es.enter_context(tc.tile_pool(name="outp", bufs=4))

        st_b = [dram.tile([128, 32], F32, tag=f"stb{b}", name=f"stb{b}")
                for b in range(B)]
        st_g = [dram.tile([128, 32], F32, tag=f"stg{b}", name=f"stg{b}",
                          addr_space="Shared")
                for b in range(B)]

        cos_sb = constp.tile([128, R], F32, tag="cos")
        sin_sb = constp.tile([128, R], F32, tag="sin")
        perm_sb = constp.tile([128, 128], BF16, tag="perm")
        ident_sb = constp.tile([128, 128], BF16, tag="ident")
        eps_sb = constp.tile([128, 1], F32, tag="eps")
        nc.vector.memset(eps_sb[:], LN_EPS)

        cq_sb = ck_sb = bv_sb = gam_sb = bet_sb = None
        if has_bqk:
            cq_sb = constp.tile([128, R], F32, tag="cq")
            ck_sb = constp.tile([128, R], F32, tag="ck")
            for hf in range(2):
                sl = slice(hf * 2048, (hf + 1) * 2048)
                nc.sync.dma_start(cq_sb[:, sl], cq_d[:, sl])
                nc.sync.dma_start(ck_sb[:, sl], ck_d[:, sl])
        if has_bv:
            bv_sb = constp.tile([128, 128], F32, tag="bvs")
            nc.sync.dma_start(bv_sb[:], bv_d[:])
        if has_gb:
            gam_sb = constp.tile([128, 128], F32, tag="gam")
            nc.sync.dma_start(gam_sb[:], gam_d[:])
            bet_sb = constp.tile([128, 128], F32, tag="bet")
            nc.sync.dma_start(bet_sb[:], bet_d[:])

        # [dh-of-pair (h0: 0-63, h1: 64-127), b*2048 + s]
        q_sb = qkp.tile([128, R], BF16, tag="q")
        k_sb = qkp.tile([128, R], BF16, tag="k")
        q1_sb = qkp.tile([64, R], BF16, tag="q1")   # partitions 64:128 -> 0:64
        k1_sb = qkp.tile([64, R], BF16, tag="k1")
        # [key-in-tile, kt*130 + hl*65 + (dh | ones)]
        v_sb = vfp.tile([128, KT * 130], BF16, tag="v")
        # [q-in-tile, tt*128 + hl*64 + dh] for row-tile tt
        attn_sb = attnp.tile([128, 32 * 128], F32, tag="attn")
        # [row, tt*2 + (sum|sumsq)]
        stats_sb = statp.tile([128, 64], F32, tag="stats")

        # ---------------- projections ----------------
        pes = ExitStack()
        with pes:
            xqp = pes.enter_context(tc.tile_pool(name="xqp", bufs=3))
            xvp = pes.enter_context(tc.tile_pool(name="xvp", bufs=2))
            wp = pes.enter_context(tc.tile_pool(name="wp", bufs=1))
            usbp = pes.enter_context(tc.tile_pool(name="usbp", bufs=4))
            stage = pes.enter_context(tc.tile_pool(name="stage", bufs=6))
            pjp = pes.enter_context(
                tc.tile_pool(name="pjp", bufs=6, space="PSUM"))
            pvp = pes.enter_context(
                tc.tile_pool(name="pvp", bufs=2, space="PSUM"))

            def load_w(t_dram, tg):
                w_sb = wp.tile([128, DC * 128], BF16, tag=tg)
                for dc in range(DC):
                    nc.sync.dma_start(w_sb[:, dc * 128:(dc + 1) * 128],
                                      t_dram[dc * 128:(dc + 1) * 128, :])
                return w_sb

            wq_sb = load_w(wq_d, "wq")
            wk_sb = load_w(wk_d, "wk")
            wv_sb = load_w(wv_d, "wv")
            nc.sync.dma_start(perm_sb[:], perm_d[:])
            nc.sync.dma_start(ident_sb[:], ident_d[:])

            def load_xseg(pool, src, seg, tg):
                t = pool.tile([128, DC * SEGW], BF16, tag=tg)
                for dc in range(DC):
                    nc.sync.dma_start(
                        t[:, dc * SEGW:(dc + 1) * SEGW],
                        src[dc * 128:(dc + 1) * 128,
                            seg * SEGW:(seg + 1) * SEGW])
                return t

            # ones in v_sb (cols kt*130 + hl*65 + 64)
            v3 = v_sb[:].rearrange("p (x e) -> p x e", e=65)
            nc.gpsimd.memset(v3[:, :, 64:65], 1.0)

            def proj_u(w_sb, xseg, seg, nm):
                ps_u = pjp.tile([128, SEGW], F32, tag="pj",
                                name=f"psu_{nm}_{seg}")
                for dc in range(DC):
                    nc.tensor.matmul(
                        ps_u[:],
                        w_sb[:, dc * 128:(dc + 1) * 128],
                        xseg[:, dc * SEGW:(dc + 1) * SEGW],
                        start=(dc == 0), stop=(dc == DC - 1))
                return ps_u

            def rope(ps_u, seg, c_sb, dst, dst1, nm):
                sl = slice(seg * SEGW, (seg + 1) * SEGW)
                u_sb = usbp.tile([128, SEGW], BF16, tag="usb",
                                 name=f"usb_{nm}_{seg}")
                nc.scalar.copy(u_sb[:], ps_u[:])                     # Act
                ps_u2 = pjp.tile([128, SEGW], F32, tag="pj",
                                 name=f"psu2_{nm}_{seg}")
                nc.tensor.matmul(ps_u2[:], perm_sb[:], u_sb[:],
                                 start=True, stop=True)
                t1 = stage.tile([128, SEGW], F32, tag="st",
                                name=f"t1_{nm}_{seg}")
                nc.vector.tensor_mul(t1[:], ps_u[:], cos_sb[:, sl])  # DVE
                t2 = stage.tile([128, SEGW], F32, tag="st",
                                name=f"t2_{nm}_{seg}")
                nc.vector.tensor_mul(t2[:], ps_u2[:], sin_sb[:, sl])  # DVE
                if c_sb is None:
                    nc.gpsimd.tensor_tensor(dst[:, sl], t1[:], t2[:],
                                            ALU.add)                 # Pool
                else:
                    t3 = stage.tile([128, SEGW], F32, tag="st",
                                    name=f"t3_{nm}_{seg}")
                    nc.gpsimd.tensor_tensor(t3[:], t1[:], t2[:], ALU.add)
                    nc.gpsimd.tensor_tensor(dst[:, sl], t3[:], c_sb[:, sl],
                                            ALU.add)
                nc.sync.dma_start(dst1[:, sl], dst[64:128, sl])

            def v_proj(xvseg, seg):
                for j in range(4):
                    kt = seg * 4 + j
                    ps_v = pvp.tile([128, 128], F32, tag="pv",
                                    name=f"psv_{kt}")
                    for dc in range(DC):
                        nc.tensor.matmul(
                            ps_v[:],
                            xvseg[:, dc * SEGW + j * 128:
                                  dc * SEGW + (j + 1) * 128],
                            wv_sb[:, dc * 128:(dc + 1) * 128],
                            start=(dc == 0), stop=(dc == DC - 1))
                    dstv = v_sb[:, kt * 130:(kt + 1) * 130].rearrange(
                        "p (h e) -> p h e", e=65)[:, :, 0:64]
                    srcv = ps_v[:].rearrange("p (h d) -> p h d", d=64)
                    if has_bv:
                        bvv = bv_sb[:].rearrange("p (h d) -> p h d", d=64)
                        nc.vector.tensor_add(dstv, srcv, bvv)
                    else:
                        nc.vector.tensor_copy(dstv, srcv)            # DVE

            for seg in range(NSEG):
                xseg = load_xseg(xqp, xqT, seg, "xq")
                if seg == 0:
                    for hf in range(2):
                        sl = slice(hf * 2048, (hf + 1) * 2048)
                        nc.sync.dma_start(cos_sb[:, sl], cos_d[:, sl])
                        nc.sync.dma_start(sin_sb[:, sl], sin_d[:, sl])
                ps_q = proj_u(wq_sb, xseg, seg, "q")
                ps_k = proj_u(wk_sb, xseg, seg, "k")
                rope(ps_q, seg, cq_sb, q_sb, q1_sb, "q")
                rope(ps_k, seg, ck_sb, k_sb, k1_sb, "k")
            for seg in range(NSEG):
                xvseg = load_xseg(xvp, xvT, seg, "xv")
                v_proj(xvseg, seg)

        # ---------------- attention ----------------
        aes = ExitStack()
        with aes:
            scp = aes.enter_context(
                tc.tile_pool(name="scp", bufs=2, space="PSUM"))
            avp = aes.enter_context(
                tc.tile_pool(name="avp", bufs=2, space="PSUM"))
            trp = aes.enter_context(
                tc.tile_pool(name="trp", bufs=2, space="PSUM"))
            atsb = aes.enter_context(tc.tile_pool(name="atsb", bufs=2))
            recp = aes.enter_context(tc.tile_pool(name="recp", bufs=2))
            sqp = aes.enter_context(tc.tile_pool(name="sqp", bufs=2))

            # unit = (b, qb, hl); per batch, qb-major with hl inner so a
            # (qb, both heads) pair completes 4 row-tiles of attn_sb.
            units = [(b, qb, hl)
                     for b in range(B) for qb in range(NQB) for hl in range(2)]

            def stage1(u):
                """scores + exp -> pt tile (returned)."""
                b, qb, hl = u
                ksrc = k_sb if hl == 0 else k1_sb
                qsrc = q_sb if hl == 0 else q1_sb
                qsl = slice(b * S + qb * 512, b * S + (qb + 1) * 512)
                pt = ptp.tile([128, 16 * 512], BF16, tag="pt",
                              name=f"pt_{b}_{qb}_{hl}")
                for st in range(8):
                    ps_s = scp.tile([128, 1024], F32, tag="sc",
                                    name=f"sc_{b}_{qb}_{hl}_{st}")
                    for j in range(2):
                        ktb = st * 2 + j
                        nc.tensor.matmul(
                            ps_s[:, j * 512:(j + 1) * 512],
                            ksrc[0:64, b * S + ktb * 128:
                                 b * S + (ktb + 1) * 128],
                            qsrc[0:64, qsl],
                            start=True, stop=True)
                    nc.scalar.activation(
                        pt[:, st * 1024:(st + 1) * 1024], ps_s[:],
                        AF.Exp, scale=0.125)
                return pt

            def stage2(u, pt):
                """AV + transpose + normalize -> attn_sb columns."""
                b, qb, hl = u
                aT = avp.tile([65, 512], F32, tag="av",
                              name=f"aT_{b}_{qb}_{hl}")
                for ktb in range(16):
                    kt = b * 16 + ktb
                    nc.tensor.matmul(
                        aT[:],
                        v_sb[:, kt * 130 + hl * 65: kt * 130 + hl * 65 + 65],
                        pt[:, ktb * 512:(ktb + 1) * 512],
                        start=(ktb == 0), stop=(ktb == 15))
                aT_sb = atsb.tile([65, 512], BF16, tag="ats",
                                  name=f"ats_{b}_{qb}_{hl}")
                nc.vector.tensor_copy(aT_sb[:], aT[:])               # DVE
                tr = trp.tile([128, 264], BF16, tag="tr",
                              name=f"tr_{b}_{qb}_{hl}")
                for t in range(4):
                    nc.tensor.transpose(
                        tr[:, t * 66: t * 66 + 65],
                        aT_sb[:, t * 128:(t + 1) * 128],
                        ident_sb[0:65, 0:65])
                tr_sb = atsb.tile([128, 264], BF16, tag="trs",
                                  name=f"trs_{b}_{qb}_{hl}")
                nc.vector.tensor_copy(tr_sb[:], tr[:])               # DVE 2x
                rec = recp.tile([128, 4], F32, tag="rec",
                                name=f"rec_{b}_{qb}_{hl}")
                nc.vector.reciprocal(rec[:], tr_sb[:, 64::66])       # DVE
                for t in range(4):
                    tt = b * 16 + qb * 4 + t
                    nc.gpsimd.tensor_scalar(                         # Pool
                        attn_sb[:, tt * 128 + hl * 64:
                                tt * 128 + hl * 64 + 64],
                        tr_sb[:, t * 66: t * 66 + 64],
                        rec[:, t: t + 1], None, ALU.mult)

            def stats_qb(b, qb):
                for t in range(4):
                    tt = b * 16 + qb * 4 + t
                    at = attn_sb[:, tt * 128:(tt + 1) * 128]
                    nc.vector.reduce_sum(
                        stats_sb[:, 2 * tt: 2 * tt + 1], at, axis=AX.X)
                    sq = sqp.tile([128, 128], F32, tag="sq",
                                  name=f"sq_{tt}")
                    nc.vector.tensor_tensor_reduce(
                        sq[:], at, at, 1.0, 0.0, ALU.mult, ALU.add,
                        accum_out=stats_sb[:, 2 * tt + 1: 2 * tt + 2])

            def stats_flush(b):
                nc.sync.dma_start(st_b[b][:],
                                  stats_sb[:, b * 32:(b + 1) * 32])
                nc.gpsimd.collective_compute(
                    "AllReduce", ALU.add,
                    ins=[st_b[b][:].opt()], outs=[st_g[b][:].opt()],
                    replica_groups=[list(range(NC))])

            def ln_half(b):
                tot = lnp.tile([128, 32], F32, tag="tot", name=f"tot{b}")
                nc.sync.dma_start(tot[:], st_g[b][:])
                nmu = lnp.tile([128, 16], F32, tag="nmu", name=f"nmu{b}")
                nc.vector.tensor_scalar_mul(nmu[:], tot[:, 0::2], -1.0 / D)
                ex2 = lnp.tile([128, 16], F32, tag="ex2", name=f"ex2{b}")
                nc.vector.tensor_scalar_mul(ex2[:], tot[:, 1::2], 1.0 / D)
                var = lnp.tile([128, 16], F32, tag="var", name=f"var{b}")
                nc.vector.tensor_tensor(var[:], nmu[:], nmu[:], ALU.mult)
                nc.vector.tensor_tensor(var[:], ex2[:], var[:], ALU.subtract)
                std = lnp.tile([128, 16], F32, tag="std", name=f"std{b}")
                nc.scalar.activation(std[:], var[:], AF.Sqrt,
                                     bias=eps_sb[:])                 # Act
                rstd = lnp.tile([128, 16], F32, tag="rstd", name=f"rstd{b}")
                nc.vector.reciprocal(rstd[:], std[:])
                mrs = lnp.tile([128, 16], F32, tag="mrs", name=f"mrs{b}")
                nc.vector.tensor_tensor(mrs[:], nmu[:], rstd[:], ALU.mult)
                for t in range(16):
                    tt = b * 16 + t
                    o_sb = outp.tile([128, 128], F32, tag="o",
                                     name=f"o_{tt}")
                    nc.vector.tensor_scalar(                         # DVE
                        o_sb[:], attn_sb[:, tt * 128:(tt + 1) * 128],
                        rstd[:, t: t + 1], mrs[:, t: t + 1],
                        ALU.mult, ALU.add)
                    if has_gb:
                        nc.vector.tensor_tensor(
                            o_sb[:], o_sb[:], gam_sb[:], ALU.mult)
                        nc.vector.tensor_tensor(
                            o_sb[:], o_sb[:], bet_sb[:], ALU.add)
                    nc.sync.dma_start(out_d[tt * 128:(tt + 1) * 128, :],
                                      o_sb[:])

            pend = None
            for i, u in enumerate(units):
                pt = stage1(u)
                if pend is not None:
                    stage2(*pend)
                    pb, pqb, phl = pend[0]
                    if phl == 1:
                        stats_qb(pb, pqb)
                        if pqb == NQB - 1:
                            stats_flush(pb)
                if i == len(units) - 1:
                    ln_half(0)
                pend = (u, pt)
            stage2(*pend)
            stats_qb(B - 1, NQB - 1)
            stats_flush(B - 1)
            ln_half(1)

    nc.compile()
    return nc


_CACHE: dict = {}
LAST_EXEC_NS = None


def _rope_tables():
    half = DH // 2
    inv_freq = 1.0 / (ROPE_BASE ** (np.arange(half, dtype=np.float32) / half))
    t = np.arange(S, dtype=np.float32)
    freqs = t[:, None] * inv_freq[None, :]
    emb = np.concatenate([freqs, freqs], axis=-1)          # [S, DH]
    return np.cos(emb).astype(np.float32), np.sin(emb).astype(np.float32)


def prep_flags(inputs):
    b_qk = np.asarray(inputs["b_qk"], dtype=np.float32)
    b_v = np.asarray(inputs["b_v"], dtype=np.float32)
    gamma = np.asarray(inputs["ln_gamma"], dtype=np.float32)
    beta = np.asarray(inputs["ln_beta"], dtype=np.float32)
    return (bool(np.any(b_qk)), bool(np.any(b_v)),
            bool(np.any(gamma != 1.0) or np.any(beta != 0.0)))


def _perm_mat():
    Pm = np.zeros((128, 128), np.float32)
    for i in range(64):
        Pm[2 * i + 1, 2 * i] = -1.0
        Pm[2 * i, 2 * i + 1] = 1.0
    return Pm


def _prep_in_maps(inputs, flags):
    x_qk = np.asarray(inputs["x_qk"], dtype=np.float32)
    x_v = np.asarray(inputs["x_v"], dtype=np.float32)
    W_qk = np.asarray(inputs["W_qk"], dtype=np.float32)
    b_qk = np.asarray(inputs["b_qk"], dtype=np.float32)
    W_v = np.asarray(inputs["W_v"], dtype=np.float32)
    b_v = np.asarray(inputs["b_v"], dtype=np.float32)
    gamma = np.asarray(inputs["ln_gamma"], dtype=np.float32)
    beta = np.asarray(inputs["ln_beta"], dtype=np.float32)

    Pm = _perm_mat()
    Pm64 = Pm[:DH, :DH]
    cos_all, sin_all = _rope_tables()          # [S, 64]
    cos_in = np.ascontiguousarray(np.tile(cos_all.T, (2, 2)))  # [128, 4096]
    sin_in = np.ascontiguousarray(np.tile(sin_all.T, (2, 2)))

    Wq = W_qk[:, :D]
    Wk = W_qk[:, D:]
    bq = b_qk[:D].reshape(H, DH)
    bk = b_qk[D:].reshape(H, DH)
    bq2 = bq @ Pm64
    bk2 = bk @ Pm64

    xqT_np = np.ascontiguousarray(
        x_qk.reshape(R, D).T.astype(NP_BF16))
    xvT_np = np.ascontiguousarray(
        x_v.reshape(R, D).T.astype(NP_BF16))
    perm_np = np.ascontiguousarray(Pm.astype(NP_BF16))
    ident_np = np.ascontiguousarray(np.eye(128, dtype=NP_BF16))

    in_maps = []
    for c in range(NC):
        cols = slice(c * 128, (c + 1) * 128)
        m = {
            "xqT": xqT_np, "xvT": xvT_np,
            "wq": np.ascontiguousarray(Wq[:, cols].astype(NP_BF16)),
            "wk": np.ascontiguousarray(Wk[:, cols].astype(NP_BF16)),
            "wv": np.ascontiguousarray(W_v[:, cols].astype(NP_BF16)),
            "perm": perm_np, "ident": ident_np,
            "cos": cos_in, "sin": sin_in,
        }
        if flags[0]:
            # additive post-RoPE bias tables for this head pair
            def fold(bh, bh2):
                rows = [bh[2 * c + hl][:, None] * cos_all.T
                        + bh2[2 * c + hl][:, None] * sin_all.T
                        for hl in range(2)]          # each [64, S]
                return np.ascontiguousarray(
                    np.tile(np.vstack(rows), (1, 2)).astype(np.float32))
            m["cq"] = fold(bq, bq2)
            m["ck"] = fold(bk, bk2)
        if flags[1]:
            m["bv"] = np.ascontiguousarray(np.broadcast_to(
                b_v[c * 128:(c + 1) * 128], (128, 128)).astype(np.float32))
        if flags[2]:
            m["gamma"] = np.ascontiguousarray(np.broadcast_to(
                gamma[c * 128:(c + 1) * 128], (128, 128)).astype(np.float32))
            m["beta"] = np.ascontiguousarray(np.broadcast_to(
                beta[c * 128:(c + 1) * 128], (128, 128)).astype(np.float32))
        in_maps.append(m)
    return in_maps


def kernel(**inputs):
    flags = prep_flags(inputs)
    if flags not in _CACHE:
        _CACHE[flags] = _build(flags)
    nc = _CACHE[flags]
    in_maps = _prep_in_maps(inputs, flags)
    res = bass_utils.run_bass_kernel_spmd(
        nc, in_maps, core_ids=list(range(NC)))
    global LAST_EXEC_NS
    LAST_EXEC_NS = res.exec_time_ns
    out = np.empty((R, D), np.float32)
    for c in range(NC):
        out[:, c * 128:(c + 1) * 128] = np.asarray(
            res.results[c]["out"], dtype=np.float32)
    return out.reshape(B, S, D)
